# revision 11
# baseline (speedup 1.0000x reference)
"""Trainium2 Bass kernel for nn_Conv1dAttention.

Math (per sample):
  q,k,v,pe = lrelu(bn(conv1d(x, W_p)))           # [C=128, L=2048], Cin=64, K=3
  S = q^T k                                      # [L, L]
  P = softmax_rows(S)                            # softmax over last axis
  out = v @ P + pe                               # [C, L]

Sharding: data-parallel over batch B=16 across 8 NeuronCores (2 samples/core).
Same NEFF on all cores, per-core input shards, no collectives.

Design notes:
  - BN (uses given mean/var, not data stats) is folded into conv weights and
    bias on the host. Bias is injected via an appended ones-row in the im2col
    tile, so conv+bias is pure matmul.
  - im2col: contraction 192 = two chunks: chunk1 = 128 rows (k=0 shifted,
    k=1 center), chunk2 = 65 rows (k=2 shifted + ones row for bias).
  - Q, K, PE computed in [c, l] layout; V directly transposed [l, c].
  - bf16 matmul operands everywhere; PSUM accumulation fp32.
  - ScalarE (exp) is the roofline engine (~73us of EXP per core); the
    schedule keeps it saturated: prelude computes only what the first S
    block needs (k all, q quarter 0), everything else is deadline-paced
    PE filler inside the two attention phases.
  - Z (softmax row sums) via a DVE tensor_scalar copy-with-accumulator over
    the bf16 P tile (4x mode), keeping Scalar at its pure-exp floor.
  - LReLU drains: one DVE scalar_tensor_tensor op reading PSUM twice
    (max(0.3*y, y)); prelude drains use ScalarE's native Lrelu (idle then).
  - Output stored bf16 (halves the out DMA), cast to fp32 on host.
  - PSUM: 4 banks out accumulator; 4 banks rotating S/conv tiles.
"""

import sys

if "/opt/trn_rl_repo" not in sys.path:
    sys.path.insert(0, "/opt/trn_rl_repo")

from contextlib import ExitStack

import ml_dtypes
import numpy as np

import concourse.bass as bass
import concourse.tile as tile
from concourse import bacc, mybir
from concourse.bass_utils import run_bass_kernel_spmd

B, CIN, COUT, KW, L = 16, 64, 128, 3, 2048
NCORES = 8
BP = B // NCORES  # samples per core
EPS = 1e-5
SLOPE = 0.3
F32 = mybir.dt.float32
BF16 = mybir.dt.bfloat16
NB = L // 128  # 16 a-blocks
HALF = 1024
NWARM = 28

_CACHE = {}


def _body(ctx, tc, x, w1, w2, zc, onesrow, out):
    nc = tc.nc
    amax = mybir.AluOpType.max
    mult = mybir.AluOpType.mult
    Exp = mybir.ActivationFunctionType.Exp
    Prelu = mybir.ActivationFunctionType.Prelu

    wpool = ctx.enter_context(tc.tile_pool(name="wpool", bufs=1))
    xpool = ctx.enter_context(tc.tile_pool(name="xpool", bufs=2))
    apool = ctx.enter_context(tc.tile_pool(name="apool", bufs=2))
    ppool = ctx.enter_context(tc.tile_pool(name="ppool", bufs=3))
    opool = ctx.enter_context(tc.tile_pool(name="opool", bufs=2))
    vpool = ctx.enter_context(tc.tile_pool(name="vpool", bufs=3))
    zpool = ctx.enter_context(tc.tile_pool(name="zpool", bufs=4))
    lpool = ctx.enter_context(tc.tile_pool(name="lpool", bufs=2))
    psA = ctx.enter_context(tc.tile_pool(name="psA", bufs=2, space="PSUM"))
    psO = ctx.enter_context(tc.tile_pool(name="psO", bufs=1, space="PSUM"))

    # Weight DMAs issued on the gpsimd queue: they trigger in parallel with
    # the x-im2col triggers on the sync queue, so neither serializes behind
    # the other at engine boot.
    w1_t, w2_t = {}, {}
    for p in "qkvp":
        w1_t[p] = wpool.tile([128, COUT], BF16, tag=f"w1{p}", name=f"w1{p}")
        nc.gpsimd.dma_start(out=w1_t[p][:, :], in_=w1[p][:, :])
        w2_t[p] = wpool.tile([CIN + 1, COUT], BF16, tag=f"w2{p}", name=f"w2{p}")
        nc.gpsimd.dma_start(out=w2_t[p][:, :], in_=w2[p][:, :])

    def emit_xs(s, split=False):
        # im2col tiles.
        # xs1 rows 0-63  = x[cin, l-1]  (k=0), rows 64-127 = x[cin, l] (k=1)
        # xs2 rows 0-63  = x[cin, l+1]  (k=2), row 64 = ones (bias)
        # split=True: column-halved transfers, xs2 pieces on separate engine
        # queues so all triggers issue in parallel at boot.
        e2 = nc.scalar if split else nc.sync
        e3 = nc.gpsimd if split else nc.sync
        xs1 = xpool.tile([128, L], BF16, tag="xs1", name="xs1")
        xs2 = xpool.tile([CIN + 1, L], BF16, tag="xs2", name="xs2")
        if split:
            nc.sync.dma_start(out=xs1[0:CIN, 1:HALF], in_=x[s, :, 0 : HALF - 1])
            nc.sync.dma_start(out=xs1[CIN:128, 0:HALF], in_=x[s, :, 0:HALF])
            nc.sync.dma_start(out=xs1[0:CIN, 0:1], in_=zc[:, :])
            e3.dma_start(out=xs1[0:CIN, HALF:L], in_=x[s, :, HALF - 1 : L - 1])
            e3.dma_start(out=xs1[CIN:128, HALF:L], in_=x[s, :, HALF:L])
            e2.dma_start(out=xs2[0:CIN, 0:HALF], in_=x[s, :, 1 : HALF + 1])
            e2.dma_start(out=xs2[CIN : CIN + 1, :], in_=onesrow[:, :])
            e2.dma_start(out=xs2[0:CIN, HALF : L - 1], in_=x[s, :, HALF + 1 : L])
            e2.dma_start(out=xs2[0:CIN, L - 1 : L], in_=zc[:, :])
        else:
            nc.sync.dma_start(out=xs1[0:CIN, 1:L], in_=x[s, :, 0 : L - 1])
            nc.sync.dma_start(out=xs1[0:CIN, 0:1], in_=zc[:, :])
            nc.sync.dma_start(out=xs1[CIN:128, 0:L], in_=x[s, :, :])
            nc.sync.dma_start(out=xs2[0:CIN, 0 : L - 1], in_=x[s, :, 1:L])
            nc.sync.dma_start(out=xs2[0:CIN, L - 1 : L], in_=zc[:, :])
            nc.sync.dma_start(out=xs2[CIN : CIN + 1, :], in_=onesrow[:, :])
        return xs1, xs2

    def lrelu_drain(dst_ap, ps_ap, mode):
        # lrelu(y) = max(y, slope*y)
        if mode == "scalar":
            # prelude only: ScalarE native Prelu, one op, DVE untouched.
            # (Lrelu's immediate alpha is ignored by the HW table; Prelu
            # reads the per-partition alpha AP correctly.)
            nc.scalar.activation(dst_ap, ps_ap, Prelu, alpha=alpha_t[:, :])
        else:
            # DVE 2-op: psum->sbuf bf16 copy (2x mode), then max(0.3*y, y).
            # (A single op reading PSUM twice is rejected: one PSUM read port.)
            w = ps_ap.free_size()
            yb = lpool.tile([128, w], BF16, tag="yb", name="yb")
            nc.vector.tensor_scalar_mul(yb[:, :], ps_ap, 1.0)
            nc.vector.scalar_tensor_tensor(
                dst_ap, yb[:, :], SLOPE, yb[:, :], op0=mult, op1=amax
            )

    def conv_q(xs1, xs2, p, dst, q, mode):
        # one [128,512] quarter of a [c, l]-layout conv (short PSUM-slot hold)
        cps = psA.tile([128, 512], F32, tag="ps", name="cps")
        c0 = q * 512
        nc.tensor.matmul(
            cps[:, :], w1_t[p][:, :], xs1[:, c0 : c0 + 512], start=True, stop=False
        )
        nc.tensor.matmul(
            cps[:, :], w2_t[p][:, :], xs2[:, c0 : c0 + 512], start=False, stop=True
        )
        lrelu_drain(dst[:, c0 : c0 + 512], cps[:, :], mode)

    def vt_qgroup(xs1, xs2, vt, gh, mode):
        # 4 l-blocks of V in transposed [l, c] layout -> one [128,512] tile
        vps = psA.tile([128, 512], F32, tag="ps", name="vps")
        for i in range(4):
            blk = gh * 4 + i
            lsl = slice(blk * 128, blk * 128 + 128)
            pc = slice(i * 128, i * 128 + 128)
            nc.tensor.matmul(
                vps[:, pc], xs1[:, lsl], w1_t["v"][:, :], start=True, stop=False
            )
            nc.tensor.matmul(
                vps[:, pc], xs2[:, lsl], w2_t["v"][:, :], start=False, stop=True
            )
        lrelu_drain(vt[:, gh * 512 : (gh + 1) * 512], vps[:, :], mode)

    def make_tiles():
        q_t = apool.tile([128, L], BF16, tag="actq", name="actq")
        k_t = apool.tile([128, L], BF16, tag="actk", name="actk")
        pe_t = apool.tile([128, L], BF16, tag="actp", name="actp")
        vt = apool.tile([128, L], BF16, tag="vt", name="vt")
        return q_t, k_t, pe_t, vt

    def attn_body(tiles, blk):
        """S matmuls + exp + normalization prep for one 128-row block.

        Z (softmax row sums) alternates per block between ScalarE's
        activation accumulator (costs 2 accumulator-read ops on the exp
        engine) and a fused DVE op over the bf16 P tile, spreading the Z
        cost across both engines."""
        q_t, k_t, pe_t, vt = tiles
        z_on_dve = blk % 2 == 1
        pblk = ppool.tile([128, L], BF16, tag="pblk", name="pblk")
        zz = zpool.tile([128, 2], F32, tag="zz", name="zz")
        for h in range(2):
            sps = psA.tile([128, HALF], F32, tag="ps", name="sps")
            for n in range(2):
                c0 = h * HALF + n * 512
                nc.tensor.matmul(
                    sps[:, n * 512 : n * 512 + 512],
                    q_t[:, blk * 128 : blk * 128 + 128],
                    k_t[:, c0 : c0 + 512],
                    start=True,
                    stop=True,
                )
            nc.scalar.activation(
                pblk[:, h * HALF : (h + 1) * HALF],
                sps[:, :],
                Exp,
                accum_out=None if z_on_dve else zz[:, h : h + 1],
            )
        z = zpool.tile([128, 1], F32, tag="z", name="z")
        if z_on_dve:
            zscr = lpool.tile([128, HALF], BF16, tag="zscr", name="zscr")
            nc.vector.scalar_tensor_tensor(
                zscr[:, :],
                pblk[:, 0:HALF],
                1.0,
                pblk[:, HALF:L],
                op0=mult,
                op1=mybir.AluOpType.add,
                accum_out=z[:, :],
            )
        else:
            nc.vector.tensor_tensor(
                z[:, :], zz[:, 0:1], zz[:, 1:2], mybir.AluOpType.add
            )
        r = zpool.tile([128, 1], F32, tag="r", name="r")
        nc.vector.reciprocal(r[:, :], z[:, :])
        vts = vpool.tile([128, 128], BF16, tag="vts", name="vts")
        nc.vector.tensor_scalar_mul(
            vts[:, :], vt[:, blk * 128 : blk * 128 + 128], r[:, :]
        )
        return pblk, vts

    def out_mms(out_ps, pblk, vts, blk):
        for n in range(4):
            nc.tensor.matmul(
                out_ps[:, n * 512 : n * 512 + 512],
                vts[:, :],
                pblk[:, n * 512 : n * 512 + 512],
                start=(blk == 0),
                stop=(blk == NB - 1),
            )

    def finish_sample(tiles, out_ps, s):
        # chunked: (psum + pe) -> bf16, DMA out per 512 cols
        pe_t = tiles[2]
        outs = opool.tile([128, L], BF16, tag="outs", name="outs")
        for n in range(4):
            cols = slice(n * 512, (n + 1) * 512)
            nc.vector.scalar_tensor_tensor(
                outs[:, cols], out_ps[:, cols], 1.0, pe_t[:, cols],
                op0=mult, op1=mybir.AluOpType.add,
            )
            nc.sync.dma_start(out=out[s, :, cols], in_=outs[:, cols])

    def attention_phase(tiles, out_ps, queue):
        """Software-pipelined over NB blocks. PE issue order per iteration:
        S(blk) matmuls first (feeding ScalarE), then out(blk-1) matmuls,
        then deadline-due conv filler units. `queue` = [(deadline, thunk)]
        sorted by deadline; a unit with deadline d issues at the END of
        iteration d at the latest."""
        qi = 0
        pending = None
        for blk in range(NB):
            pblk, vts = attn_body(tiles, blk)
            if pending is not None:
                out_mms(out_ps, *pending)
            pending = (pblk, vts, blk)
            while qi < len(queue) and queue[qi][0] <= blk:
                queue[qi][1]()
                qi += 1
        while qi < len(queue):
            queue[qi][1]()
            qi += 1
        out_mms(out_ps, *pending)

    assert BP == 2
    # PE warm-up: dummy matmuls on a memset tile (no DMA dependency) keep
    # the PE busy from the start so the HAM clock-gate reaches 2.4 GHz
    # before the real work.
    wseed = wpool.tile([128, 128], BF16, tag="wseed", name="wseed")
    nc.gpsimd.memset(wseed[:, :], 0.001)
    alpha_t = wpool.tile([128, 1], F32, tag="alpha", name="alpha")
    nc.gpsimd.memset(alpha_t[:, :], SLOPE)
    wps = psA.tile([128, 128], F32, tag="ps", name="wps")
    for _ in range(NWARM):
        nc.tensor.matmul(
            wps[:, :], wseed[:, :], wseed[:, :], start=True, stop=True
        )
    # Prelude: only what attention block 0 needs -- all of K (moving side of
    # every S matmul) and Q quarter 0 (covers blocks 0-3). ScalarE-native
    # Lrelu drains (ScalarE is idle until the first exp).
    xs0 = emit_xs(0, split=True)
    tiles0 = make_tiles()
    q0, k0, pe0, vt0 = tiles0
    for q in range(4):
        conv_q(*xs0, "k", k0, q, "scalar")
    conv_q(*xs0, "q", q0, 0, "scalar")
    vt_qgroup(*xs0, vt0, 0, "scalar")
    # Phase B: sample-0 attention with remaining conv work deadline-dripped.
    # vt0 group g is read by out_mms(g*4) issued in iteration g*4+1, so its
    # deadline is g*4. q0 quarter j feeds S(4j) issued at iter 4j: deadline
    # 4j-1. Sample-1 q/k/vt group 0 are needed at phase-C start.
    xs1_ = emit_xs(1)
    tiles1 = make_tiles()
    q1, k1, pe1, vt1 = tiles1
    # Deadline rule (fillers issue at END of iteration d): a vt group g or
    # q quarter j consumed at iteration 4g must have deadline <= 4g-1, since
    # DVE/PE execute in issue order (a later-issued producer would deadlock).
    queueB = [
        (3, lambda: conv_q(*xs0, "q", q0, 1, "bf")),
        (3, lambda: vt_qgroup(*xs0, vt0, 1, "bf")),
        (5, lambda: conv_q(*xs0, "p", pe0, 0, "bf")),
        (7, lambda: conv_q(*xs0, "q", q0, 2, "bf")),
        (7, lambda: vt_qgroup(*xs0, vt0, 2, "bf")),
        (9, lambda: conv_q(*xs0, "p", pe0, 1, "bf")),
        (10, lambda: conv_q(*xs1_, "k", k1, 0, "bf")),
        (11, lambda: conv_q(*xs0, "q", q0, 3, "bf")),
        (11, lambda: vt_qgroup(*xs0, vt0, 3, "bf")),
        (12, lambda: conv_q(*xs1_, "k", k1, 1, "bf")),
        (13, lambda: conv_q(*xs0, "p", pe0, 2, "bf")),
        (13, lambda: conv_q(*xs1_, "k", k1, 2, "bf")),
        (14, lambda: conv_q(*xs1_, "k", k1, 3, "bf")),
        (14, lambda: conv_q(*xs0, "p", pe0, 3, "bf")),
        (15, lambda: conv_q(*xs1_, "q", q1, 0, "bf")),
        (15, lambda: vt_qgroup(*xs1_, vt1, 0, "bf")),
    ]
    queueC = [
        (3, lambda: conv_q(*xs1_, "q", q1, 1, "bf")),
        (3, lambda: vt_qgroup(*xs1_, vt1, 1, "bf")),
        (5, lambda: conv_q(*xs1_, "p", pe1, 0, "bf")),
        (7, lambda: conv_q(*xs1_, "q", q1, 2, "bf")),
        (7, lambda: vt_qgroup(*xs1_, vt1, 2, "bf")),
        (9, lambda: conv_q(*xs1_, "p", pe1, 1, "bf")),
        (11, lambda: conv_q(*xs1_, "q", q1, 3, "bf")),
        (11, lambda: vt_qgroup(*xs1_, vt1, 3, "bf")),
        (13, lambda: conv_q(*xs1_, "p", pe1, 2, "bf")),
        (14, lambda: conv_q(*xs1_, "p", pe1, 3, "bf")),
    ]
    out_ps0 = psO.tile([128, L], F32, tag="ops", name="out_ps0")
    attention_phase(tiles0, out_ps0, queueB)
    finish_sample(tiles0, out_ps0, 0)
    out_ps1 = psO.tile([128, L], F32, tag="ops", name="out_ps1")
    attention_phase(tiles1, out_ps1, queueC)
    finish_sample(tiles1, out_ps1, 1)


def build():
    nc = bacc.Bacc("TRN2", target_bir_lowering=False, debug=False)
    x_d = nc.dram_tensor("x", [BP, CIN, L], BF16, kind="ExternalInput")
    w1_d, w2_d = {}, {}
    for p in "qkvp":
        w1_d[p] = nc.dram_tensor(f"w1_{p}", [128, COUT], BF16, kind="ExternalInput")
        w2_d[p] = nc.dram_tensor(f"w2_{p}", [CIN + 1, COUT], BF16, kind="ExternalInput")
    zc_d = nc.dram_tensor("zc", [CIN, 1], BF16, kind="ExternalInput")
    ones_d = nc.dram_tensor("onesrow", [1, L], BF16, kind="ExternalInput")
    out_d = nc.dram_tensor("out", [BP, COUT, L], BF16, kind="ExternalOutput")

    with tile.TileContext(nc) as tc, ExitStack() as ctx:
        _body(
            ctx,
            tc,
            x_d.ap(),
            {p: w1_d[p].ap() for p in "qkvp"},
            {p: w2_d[p].ap() for p in "qkvp"},
            zc_d.ap(),
            ones_d.ap(),
            out_d.ap(),
        )
    nc.compile()
    return nc


def _fold_weights(w, b, gamma, beta, mean, var):
    """Fold BN affine (fixed mean/var) into conv weights; return im2col chunks."""
    w = np.asarray(w, np.float64)
    scale = np.asarray(gamma, np.float64) / np.sqrt(np.asarray(var, np.float64) + EPS)
    shift = np.asarray(beta, np.float64) - np.asarray(mean, np.float64) * scale
    wf = w * scale[:, None, None]  # [COUT, CIN, K]
    bf = np.asarray(b, np.float64) * scale + shift
    w1 = np.empty((128, COUT), np.float32)
    w1[0:CIN] = wf[:, :, 0].T
    w1[CIN:128] = wf[:, :, 1].T
    w2 = np.empty((CIN + 1, COUT), np.float32)
    w2[0:CIN] = wf[:, :, 2].T
    w2[CIN] = bf
    return w1, w2


def _get_nc():
    if "nc" not in _CACHE:
        _CACHE["nc"] = build()
    return _CACHE["nc"]


def make_in_maps(inputs):
    bf = ml_dtypes.bfloat16
    x = np.ascontiguousarray(np.asarray(inputs["x"], np.float32).astype(bf))
    folded = {}
    for p in "qkvp":
        key = p if p != "p" else "pe"
        folded[p] = _fold_weights(
            inputs[f"{key}_w"],
            inputs[f"{key}_b"],
            inputs[f"{key}_gamma"],
            inputs[f"{key}_beta"],
            inputs[f"{key}_mean"],
            inputs[f"{key}_var"],
        )
    in_maps = []
    for i in range(NCORES):
        m = {"x": np.ascontiguousarray(x[i * BP : (i + 1) * BP])}
        for p in "qkvp":
            m[f"w1_{p}"] = folded[p][0].astype(bf)
            m[f"w2_{p}"] = folded[p][1].astype(bf)
        m["zc"] = np.zeros((CIN, 1), bf)
        m["onesrow"] = np.ones((1, L), bf)
        in_maps.append(m)
    return in_maps


def kernel(**inputs):
    nc = _get_nc()
    in_maps = make_in_maps(inputs)
    res = run_bass_kernel_spmd(nc, in_maps, core_ids=list(range(NCORES)))
    out = np.concatenate([res.results[i]["out"] for i in range(NCORES)], axis=0)
    return out.astype(np.float32)


if __name__ == "__main__":
    rng = np.random.default_rng(0)
    ins = {"x": rng.standard_normal((B, CIN, L), dtype=np.float32)}
    for p in ("q", "k", "v", "pe"):
        ins[f"{p}_w"] = (rng.standard_normal((COUT, CIN, KW)) * 0.05).astype(np.float32)
        ins[f"{p}_b"] = (rng.standard_normal(COUT) * 0.05).astype(np.float32)
        ins[f"{p}_gamma"] = rng.uniform(0.5, 1.5, COUT).astype(np.float32)
        ins[f"{p}_beta"] = (rng.standard_normal(COUT) * 0.05).astype(np.float32)
        ins[f"{p}_mean"] = (rng.standard_normal(COUT) * 0.05).astype(np.float32)
        ins[f"{p}_var"] = rng.uniform(0.5, 1.5, COUT).astype(np.float32)
    got = kernel(**ins)
    print("kernel output:", got.shape, got.dtype, np.abs(got).mean())


# revision 16
# speedup vs baseline: 1.0334x; 1.0334x over previous
"""Trainium2 Bass kernel for nn_Conv1dAttention.

Math (per sample):
  q,k,v,pe = lrelu(bn(conv1d(x, W_p)))           # [C=128, L=2048], Cin=64, K=3
  S = q^T k                                      # [L, L]
  P = softmax_rows(S)                            # softmax over last axis
  out = v @ P + pe                               # [C, L]

Sharding: data-parallel over batch B=16 across 8 NeuronCores (2 samples/core).
Same NEFF on all cores, per-core input shards, no collectives.

Design notes:
  - BN (uses given mean/var, not data stats) is folded into conv weights and
    bias on the host. Bias is injected via an appended ones-row in the im2col
    tile, so conv+bias is pure matmul.
  - im2col: contraction 192 = two chunks: chunk1 = 128 rows (k=0 shifted,
    k=1 center), chunk2 = 65 rows (k=2 shifted + ones row for bias).
  - Q, K, PE computed in [c, l] layout; V directly transposed [l, c].
  - bf16 matmul operands everywhere; PSUM accumulation fp32.
  - ScalarE (exp) is the roofline engine (~73us of EXP per core); the
    schedule keeps it saturated: prelude computes only what the first S
    block needs (k all, q quarter 0), everything else is deadline-paced
    PE filler inside the two attention phases.
  - Z (softmax row sums) via a DVE tensor_scalar copy-with-accumulator over
    the bf16 P tile (4x mode), keeping Scalar at its pure-exp floor.
  - LReLU drains: one DVE scalar_tensor_tensor op reading PSUM twice
    (max(0.3*y, y)); prelude drains use ScalarE's native Lrelu (idle then).
  - Output stored bf16 (halves the out DMA), cast to fp32 on host.
  - PSUM: 4 banks out accumulator; 4 banks rotating S/conv tiles.
"""

import sys

if "/opt/trn_rl_repo" not in sys.path:
    sys.path.insert(0, "/opt/trn_rl_repo")

from contextlib import ExitStack

import ml_dtypes
import numpy as np

import concourse.bass as bass
import concourse.tile as tile
from concourse import bacc, mybir
from concourse.bass_utils import run_bass_kernel_spmd

B, CIN, COUT, KW, L = 16, 64, 128, 3, 2048
NCORES = 8
BP = B // NCORES  # samples per core
EPS = 1e-5
SLOPE = 0.3
F32 = mybir.dt.float32
BF16 = mybir.dt.bfloat16
NB = L // 128  # 16 a-blocks
HALF = 1024
NWARM = 28

_CACHE = {}


def _body(ctx, tc, x, w1, w2, zc, onesrow, out):
    nc = tc.nc
    amax = mybir.AluOpType.max
    mult = mybir.AluOpType.mult
    Exp = mybir.ActivationFunctionType.Exp
    Prelu = mybir.ActivationFunctionType.Prelu

    wpool = ctx.enter_context(tc.tile_pool(name="wpool", bufs=1))
    xpool = ctx.enter_context(tc.tile_pool(name="xpool", bufs=2))
    apool = ctx.enter_context(tc.tile_pool(name="apool", bufs=2))
    ppool = ctx.enter_context(tc.tile_pool(name="ppool", bufs=3))
    opool = ctx.enter_context(tc.tile_pool(name="opool", bufs=2))
    vpool = ctx.enter_context(tc.tile_pool(name="vpool", bufs=3))
    zpool = ctx.enter_context(tc.tile_pool(name="zpool", bufs=4))
    lpool = ctx.enter_context(tc.tile_pool(name="lpool", bufs=2))
    psA = ctx.enter_context(tc.tile_pool(name="psA", bufs=2, space="PSUM"))
    psO = ctx.enter_context(tc.tile_pool(name="psO", bufs=1, space="PSUM"))

    # Weight DMAs on the scalar queue (hardware DGE), x-im2col on the sync
    # queue: the triggers issue in parallel at engine boot instead of
    # serializing behind one another. (gpsimd DMA = software DGE, slow.)
    w1_t, w2_t = {}, {}
    for p in "qkvp":
        w1_t[p] = wpool.tile([128, COUT], BF16, tag=f"w1{p}", name=f"w1{p}")
        nc.scalar.dma_start(out=w1_t[p][:, :], in_=w1[p][:, :])
        w2_t[p] = wpool.tile([CIN + 1, COUT], BF16, tag=f"w2{p}", name=f"w2{p}")
        nc.scalar.dma_start(out=w2_t[p][:, :], in_=w2[p][:, :])

    def emit_xs(s, split=False):
        # im2col tiles.
        # xs1 rows 0-63  = x[cin, l-1]  (k=0), rows 64-127 = x[cin, l] (k=1)
        # xs2 rows 0-63  = x[cin, l+1]  (k=2), row 64 = ones (bias)
        # split=True: column-halved transfers, xs2 pieces on separate engine
        # queues so all triggers issue in parallel at boot.
        e2 = nc.scalar if split else nc.sync
        e3 = nc.sync
        xs1 = xpool.tile([128, L], BF16, tag="xs1", name="xs1")
        xs2 = xpool.tile([CIN + 1, L], BF16, tag="xs2", name="xs2")
        if split:
            nc.sync.dma_start(out=xs1[0:CIN, 1:HALF], in_=x[s, :, 0 : HALF - 1])
            nc.sync.dma_start(out=xs1[CIN:128, 0:HALF], in_=x[s, :, 0:HALF])
            nc.sync.dma_start(out=xs1[0:CIN, 0:1], in_=zc[:, :])
            e3.dma_start(out=xs1[0:CIN, HALF:L], in_=x[s, :, HALF - 1 : L - 1])
            e3.dma_start(out=xs1[CIN:128, HALF:L], in_=x[s, :, HALF:L])
            e2.dma_start(out=xs2[0:CIN, 0:HALF], in_=x[s, :, 1 : HALF + 1])
            e2.dma_start(out=xs2[CIN : CIN + 1, :], in_=onesrow[:, :])
            e2.dma_start(out=xs2[0:CIN, HALF : L - 1], in_=x[s, :, HALF + 1 : L])
            e2.dma_start(out=xs2[0:CIN, L - 1 : L], in_=zc[:, :])
        else:
            nc.sync.dma_start(out=xs1[0:CIN, 1:L], in_=x[s, :, 0 : L - 1])
            nc.sync.dma_start(out=xs1[0:CIN, 0:1], in_=zc[:, :])
            nc.sync.dma_start(out=xs1[CIN:128, 0:L], in_=x[s, :, :])
            nc.sync.dma_start(out=xs2[0:CIN, 0 : L - 1], in_=x[s, :, 1:L])
            nc.sync.dma_start(out=xs2[0:CIN, L - 1 : L], in_=zc[:, :])
            nc.sync.dma_start(out=xs2[CIN : CIN + 1, :], in_=onesrow[:, :])
        return xs1, xs2

    def lrelu_drain(dst_ap, ps_ap, mode):
        # lrelu(y) = max(y, slope*y)
        if mode == "scalar":
            # prelude only: ScalarE native Prelu, one op, DVE untouched.
            # (Lrelu's immediate alpha is ignored by the HW table; Prelu
            # reads the per-partition alpha AP correctly.)
            nc.scalar.activation(dst_ap, ps_ap, Prelu, alpha=alpha_t[:, :])
        else:
            # DVE 2-op: psum->sbuf bf16 copy (2x mode), then max(0.3*y, y).
            # (A single op reading PSUM twice is rejected: one PSUM read port.)
            w = ps_ap.free_size()
            yb = lpool.tile([128, w], BF16, tag="yb", name="yb")
            nc.vector.tensor_scalar_mul(yb[:, :], ps_ap, 1.0)
            nc.vector.scalar_tensor_tensor(
                dst_ap, yb[:, :], SLOPE, yb[:, :], op0=mult, op1=amax
            )

    def conv_q(xs1, xs2, p, dst, q, mode):
        # one [128,512] quarter of a [c, l]-layout conv (short PSUM-slot hold)
        cps = psA.tile([128, 512], F32, tag="ps", name="cps")
        c0 = q * 512
        nc.tensor.matmul(
            cps[:, :], w1_t[p][:, :], xs1[:, c0 : c0 + 512], start=True, stop=False
        )
        nc.tensor.matmul(
            cps[:, :], w2_t[p][:, :], xs2[:, c0 : c0 + 512], start=False, stop=True
        )
        lrelu_drain(dst[:, c0 : c0 + 512], cps[:, :], mode)

    def vt_qgroup(xs1, xs2, vt, gh, mode):
        # 4 l-blocks of V in transposed [l, c] layout -> one [128,512] tile
        vps = psA.tile([128, 512], F32, tag="ps", name="vps")
        for i in range(4):
            blk = gh * 4 + i
            lsl = slice(blk * 128, blk * 128 + 128)
            pc = slice(i * 128, i * 128 + 128)
            nc.tensor.matmul(
                vps[:, pc], xs1[:, lsl], w1_t["v"][:, :], start=True, stop=False
            )
            nc.tensor.matmul(
                vps[:, pc], xs2[:, lsl], w2_t["v"][:, :], start=False, stop=True
            )
        lrelu_drain(vt[:, gh * 512 : (gh + 1) * 512], vps[:, :], mode)

    def make_tiles():
        q_t = apool.tile([128, L], BF16, tag="actq", name="actq")
        k_t = apool.tile([128, L], BF16, tag="actk", name="actk")
        pe_t = apool.tile([128, L], BF16, tag="actp", name="actp")
        vt = apool.tile([128, L], BF16, tag="vt", name="vt")
        return q_t, k_t, pe_t, vt

    def attn_body(tiles, blk):
        """S matmuls + exp + normalization prep for one 128-row block.

        Z (softmax row sums) alternates per block between ScalarE's
        activation accumulator (costs 2 accumulator-read ops on the exp
        engine) and a fused DVE op over the bf16 P tile, spreading the Z
        cost across both engines."""
        q_t, k_t, pe_t, vt = tiles
        z_on_dve = False
        pblk = ppool.tile([128, L], BF16, tag="pblk", name="pblk")
        zz = zpool.tile([128, 2], F32, tag="zz", name="zz")
        for h in range(2):
            sps = psA.tile([128, HALF], F32, tag="ps", name="sps")
            for n in range(2):
                c0 = h * HALF + n * 512
                nc.tensor.matmul(
                    sps[:, n * 512 : n * 512 + 512],
                    q_t[:, blk * 128 : blk * 128 + 128],
                    k_t[:, c0 : c0 + 512],
                    start=True,
                    stop=True,
                )
            nc.scalar.activation(
                pblk[:, h * HALF : (h + 1) * HALF],
                sps[:, :],
                Exp,
                accum_out=None if z_on_dve else zz[:, h : h + 1],
            )
        z = zpool.tile([128, 1], F32, tag="z", name="z")
        if z_on_dve:
            zscr = lpool.tile([128, HALF], BF16, tag="zscr", name="zscr")
            nc.vector.scalar_tensor_tensor(
                zscr[:, :],
                pblk[:, 0:HALF],
                1.0,
                pblk[:, HALF:L],
                op0=mult,
                op1=mybir.AluOpType.add,
                accum_out=z[:, :],
            )
        else:
            nc.vector.tensor_tensor(
                z[:, :], zz[:, 0:1], zz[:, 1:2], mybir.AluOpType.add
            )
        r = zpool.tile([128, 1], F32, tag="r", name="r")
        nc.vector.reciprocal(r[:, :], z[:, :])
        vts = vpool.tile([128, 128], BF16, tag="vts", name="vts")
        nc.vector.tensor_scalar_mul(
            vts[:, :], vt[:, blk * 128 : blk * 128 + 128], r[:, :]
        )
        return pblk, vts

    def out_mms(out_ps, pblk, vts, blk):
        for n in range(4):
            nc.tensor.matmul(
                out_ps[:, n * 512 : n * 512 + 512],
                vts[:, :],
                pblk[:, n * 512 : n * 512 + 512],
                start=(blk == 0),
                stop=(blk == NB - 1),
            )

    def finish_sample(tiles, out_ps, s):
        # chunked: (psum + pe) -> bf16, DMA out per 512 cols
        pe_t = tiles[2]
        outs = opool.tile([128, L], BF16, tag="outs", name="outs")
        for n in range(4):
            cols = slice(n * 512, (n + 1) * 512)
            nc.vector.scalar_tensor_tensor(
                outs[:, cols], out_ps[:, cols], 1.0, pe_t[:, cols],
                op0=mult, op1=mybir.AluOpType.add,
            )
            nc.sync.dma_start(out=out[s, :, cols], in_=outs[:, cols])

    def attention_phase(tiles, out_ps, queue):
        """Software-pipelined over NB blocks. PE issue order per iteration:
        S(blk) matmuls first (feeding ScalarE), then out(blk-2) matmuls
        (the 2-block lag gives the z->recip->vts DVE chain slack), then
        deadline-due conv filler units. `queue` = [(deadline, thunk)]
        sorted by deadline; a unit with deadline d issues at the END of
        iteration d at the latest."""
        qi = 0
        pending = []
        for blk in range(NB):
            pblk, vts = attn_body(tiles, blk)
            pending.append((pblk, vts, blk))
            if len(pending) > 2:
                out_mms(out_ps, *pending.pop(0))
            while qi < len(queue) and queue[qi][0] <= blk:
                queue[qi][1]()
                qi += 1
        while qi < len(queue):
            queue[qi][1]()
            qi += 1
        for p in pending:
            out_mms(out_ps, *p)

    assert BP == 2
    # PE warm-up: dummy matmuls on a memset tile (no DMA dependency) keep
    # the PE busy from the start so the HAM clock-gate reaches 2.4 GHz
    # before the real work.
    wseed = wpool.tile([128, 128], BF16, tag="wseed", name="wseed")
    nc.gpsimd.memset(wseed[:, :], 0.001)
    alpha_t = wpool.tile([128, 1], F32, tag="alpha", name="alpha")
    nc.gpsimd.memset(alpha_t[:, :], SLOPE)
    wps = psA.tile([128, 128], F32, tag="ps", name="wps")
    for _ in range(NWARM):
        nc.tensor.matmul(
            wps[:, :], wseed[:, :], wseed[:, :], start=True, stop=True
        )
    # Prelude: only what attention block 0 needs -- all of K (moving side of
    # every S matmul) and Q quarter 0 (covers blocks 0-3). ScalarE-native
    # Lrelu drains (ScalarE is idle until the first exp).
    xs0 = emit_xs(0, split=True)
    tiles0 = make_tiles()
    q0, k0, pe0, vt0 = tiles0
    for q in range(4):
        conv_q(*xs0, "k", k0, q, "scalar")
    conv_q(*xs0, "q", q0, 0, "scalar")
    vt_qgroup(*xs0, vt0, 0, "scalar")
    # Phase B: sample-0 attention with remaining conv work deadline-dripped.
    # vt0 group g is read by out_mms(g*4) issued in iteration g*4+1, so its
    # deadline is g*4. q0 quarter j feeds S(4j) issued at iter 4j: deadline
    # 4j-1. Sample-1 q/k/vt group 0 are needed at phase-C start.
    xs1_ = emit_xs(1)
    tiles1 = make_tiles()
    q1, k1, pe1, vt1 = tiles1
    # Deadline rule (fillers issue at END of iteration d): a vt group g or
    # q quarter j consumed at iteration 4g must have deadline <= 4g-1, since
    # DVE/PE execute in issue order (a later-issued producer would deadlock).
    # At most ONE filler unit per iteration: double-filler iterations were
    # measured to open ~3us ScalarE bubbles at the group boundaries.
    queueB = [
        (0, lambda: conv_q(*xs0, "p", pe0, 0, "bf")),
        (1, lambda: conv_q(*xs0, "p", pe0, 1, "bf")),
        (2, lambda: vt_qgroup(*xs0, vt0, 1, "bf")),
        (3, lambda: conv_q(*xs0, "q", q0, 1, "bf")),
        (4, lambda: conv_q(*xs0, "p", pe0, 2, "bf")),
        (5, lambda: conv_q(*xs0, "p", pe0, 3, "bf")),
        (6, lambda: vt_qgroup(*xs0, vt0, 2, "bf")),
        (7, lambda: conv_q(*xs0, "q", q0, 2, "bf")),
        (8, lambda: conv_q(*xs1_, "k", k1, 0, "bf")),
        (9, lambda: conv_q(*xs1_, "k", k1, 1, "bf")),
        (10, lambda: vt_qgroup(*xs0, vt0, 3, "bf")),
        (11, lambda: conv_q(*xs0, "q", q0, 3, "bf")),
        (12, lambda: conv_q(*xs1_, "k", k1, 2, "bf")),
        (13, lambda: conv_q(*xs1_, "k", k1, 3, "bf")),
        (14, lambda: conv_q(*xs1_, "q", q1, 0, "bf")),
        (15, lambda: vt_qgroup(*xs1_, vt1, 0, "bf")),
    ]
    queueC = [
        (2, lambda: vt_qgroup(*xs1_, vt1, 1, "bf")),
        (3, lambda: conv_q(*xs1_, "q", q1, 1, "bf")),
        (5, lambda: conv_q(*xs1_, "p", pe1, 0, "bf")),
        (6, lambda: vt_qgroup(*xs1_, vt1, 2, "bf")),
        (7, lambda: conv_q(*xs1_, "q", q1, 2, "bf")),
        (9, lambda: conv_q(*xs1_, "p", pe1, 1, "bf")),
        (10, lambda: vt_qgroup(*xs1_, vt1, 3, "bf")),
        (11, lambda: conv_q(*xs1_, "q", q1, 3, "bf")),
        (13, lambda: conv_q(*xs1_, "p", pe1, 2, "bf")),
        (14, lambda: conv_q(*xs1_, "p", pe1, 3, "bf")),
    ]
    out_ps0 = psO.tile([128, L], F32, tag="ops", name="out_ps0")
    attention_phase(tiles0, out_ps0, queueB)
    finish_sample(tiles0, out_ps0, 0)
    out_ps1 = psO.tile([128, L], F32, tag="ops", name="out_ps1")
    attention_phase(tiles1, out_ps1, queueC)
    finish_sample(tiles1, out_ps1, 1)


def build():
    nc = bacc.Bacc("TRN2", target_bir_lowering=False, debug=False)
    x_d = nc.dram_tensor("x", [BP, CIN, L], BF16, kind="ExternalInput")
    w1_d, w2_d = {}, {}
    for p in "qkvp":
        w1_d[p] = nc.dram_tensor(f"w1_{p}", [128, COUT], BF16, kind="ExternalInput")
        w2_d[p] = nc.dram_tensor(f"w2_{p}", [CIN + 1, COUT], BF16, kind="ExternalInput")
    zc_d = nc.dram_tensor("zc", [CIN, 1], BF16, kind="ExternalInput")
    ones_d = nc.dram_tensor("onesrow", [1, L], BF16, kind="ExternalInput")
    out_d = nc.dram_tensor("out", [BP, COUT, L], BF16, kind="ExternalOutput")

    with tile.TileContext(nc) as tc, ExitStack() as ctx:
        _body(
            ctx,
            tc,
            x_d.ap(),
            {p: w1_d[p].ap() for p in "qkvp"},
            {p: w2_d[p].ap() for p in "qkvp"},
            zc_d.ap(),
            ones_d.ap(),
            out_d.ap(),
        )
    nc.compile()
    return nc


def _fold_weights(w, b, gamma, beta, mean, var):
    """Fold BN affine (fixed mean/var) into conv weights; return im2col chunks."""
    w = np.asarray(w, np.float64)
    scale = np.asarray(gamma, np.float64) / np.sqrt(np.asarray(var, np.float64) + EPS)
    shift = np.asarray(beta, np.float64) - np.asarray(mean, np.float64) * scale
    wf = w * scale[:, None, None]  # [COUT, CIN, K]
    bf = np.asarray(b, np.float64) * scale + shift
    w1 = np.empty((128, COUT), np.float32)
    w1[0:CIN] = wf[:, :, 0].T
    w1[CIN:128] = wf[:, :, 1].T
    w2 = np.empty((CIN + 1, COUT), np.float32)
    w2[0:CIN] = wf[:, :, 2].T
    w2[CIN] = bf
    return w1, w2


def _get_nc():
    if "nc" not in _CACHE:
        _CACHE["nc"] = build()
    return _CACHE["nc"]


def make_in_maps(inputs):
    bf = ml_dtypes.bfloat16
    x = np.ascontiguousarray(np.asarray(inputs["x"], np.float32).astype(bf))
    folded = {}
    for p in "qkvp":
        key = p if p != "p" else "pe"
        folded[p] = _fold_weights(
            inputs[f"{key}_w"],
            inputs[f"{key}_b"],
            inputs[f"{key}_gamma"],
            inputs[f"{key}_beta"],
            inputs[f"{key}_mean"],
            inputs[f"{key}_var"],
        )
    in_maps = []
    for i in range(NCORES):
        m = {"x": np.ascontiguousarray(x[i * BP : (i + 1) * BP])}
        for p in "qkvp":
            m[f"w1_{p}"] = folded[p][0].astype(bf)
            m[f"w2_{p}"] = folded[p][1].astype(bf)
        m["zc"] = np.zeros((CIN, 1), bf)
        m["onesrow"] = np.ones((1, L), bf)
        in_maps.append(m)
    return in_maps


def kernel(**inputs):
    nc = _get_nc()
    in_maps = make_in_maps(inputs)
    res = run_bass_kernel_spmd(nc, in_maps, core_ids=list(range(NCORES)))
    out = np.concatenate([res.results[i]["out"] for i in range(NCORES)], axis=0)
    return out.astype(np.float32)


if __name__ == "__main__":
    rng = np.random.default_rng(0)
    ins = {"x": rng.standard_normal((B, CIN, L), dtype=np.float32)}
    for p in ("q", "k", "v", "pe"):
        ins[f"{p}_w"] = (rng.standard_normal((COUT, CIN, KW)) * 0.05).astype(np.float32)
        ins[f"{p}_b"] = (rng.standard_normal(COUT) * 0.05).astype(np.float32)
        ins[f"{p}_gamma"] = rng.uniform(0.5, 1.5, COUT).astype(np.float32)
        ins[f"{p}_beta"] = (rng.standard_normal(COUT) * 0.05).astype(np.float32)
        ins[f"{p}_mean"] = (rng.standard_normal(COUT) * 0.05).astype(np.float32)
        ins[f"{p}_var"] = rng.uniform(0.5, 1.5, COUT).astype(np.float32)
    got = kernel(**ins)
    print("kernel output:", got.shape, got.dtype, np.abs(got).mean())


# revision 21
# speedup vs baseline: 1.0794x; 1.0445x over previous
"""Trainium2 Bass kernel for nn_Conv1dAttention.

Math (per sample):
  q,k,v,pe = lrelu(bn(conv1d(x, W_p)))           # [C=128, L=2048], Cin=64, K=3
  S = q^T k                                      # [L, L]
  P = softmax_rows(S)                            # softmax over last axis
  out = v @ P + pe                               # [C, L]

Sharding: data-parallel over batch B=16 across 8 NeuronCores (2 samples/core).
Same NEFF on all cores, per-core input shards, no collectives.

Design notes:
  - BN (uses given mean/var, not data stats) is folded into conv weights and
    bias on the host. Bias is injected via an appended ones-row in the im2col
    tile, so conv+bias is pure matmul.
  - im2col: contraction 192 = two chunks: chunk1 = 128 rows (k=0 shifted,
    k=1 center), chunk2 = 65 rows (k=2 shifted + ones row for bias).
  - Q, K, PE computed in [c, l] layout; V directly transposed [l, c].
  - bf16 matmul operands everywhere; PSUM accumulation fp32.
  - ScalarE (exp) is the roofline engine (~73us of EXP per core); the
    schedule keeps it saturated: prelude computes only what the first S
    block needs (k all, q quarter 0), everything else is deadline-paced
    PE filler inside the two attention phases.
  - Z (softmax row sums) via a DVE tensor_scalar copy-with-accumulator over
    the bf16 P tile (4x mode), keeping Scalar at its pure-exp floor.
  - LReLU drains: one DVE scalar_tensor_tensor op reading PSUM twice
    (max(0.3*y, y)); prelude drains use ScalarE's native Lrelu (idle then).
  - Output stored bf16 (halves the out DMA), cast to fp32 on host.
  - PSUM: 4 banks out accumulator; 4 banks rotating S/conv tiles.
"""

import sys

if "/opt/trn_rl_repo" not in sys.path:
    sys.path.insert(0, "/opt/trn_rl_repo")

from contextlib import ExitStack

import ml_dtypes
import numpy as np

import concourse.bass as bass
import concourse.tile as tile
from concourse import bacc, mybir
from concourse.bass_utils import run_bass_kernel_spmd

B, CIN, COUT, KW, L = 16, 64, 128, 3, 2048
NCORES = 8
BP = B // NCORES  # samples per core
EPS = 1e-5
SLOPE = 0.3
F32 = mybir.dt.float32
BF16 = mybir.dt.bfloat16
NB = L // 128  # 16 a-blocks
HALF = 1024
NWARM = 18

_CACHE = {}


def _body(ctx, tc, x, w1, w2, zc, onesrow, out):
    nc = tc.nc
    amax = mybir.AluOpType.max
    mult = mybir.AluOpType.mult
    Exp = mybir.ActivationFunctionType.Exp
    Prelu = mybir.ActivationFunctionType.Prelu

    wpool = ctx.enter_context(tc.tile_pool(name="wpool", bufs=1))
    xpool = ctx.enter_context(tc.tile_pool(name="xpool", bufs=2))
    apool = ctx.enter_context(tc.tile_pool(name="apool", bufs=2))
    ppool = ctx.enter_context(tc.tile_pool(name="ppool", bufs=3))
    opool = ctx.enter_context(tc.tile_pool(name="opool", bufs=2))
    vpool = ctx.enter_context(tc.tile_pool(name="vpool", bufs=3))
    zpool = ctx.enter_context(tc.tile_pool(name="zpool", bufs=4))
    lpool = ctx.enter_context(tc.tile_pool(name="lpool", bufs=2))
    psA = ctx.enter_context(tc.tile_pool(name="psA", bufs=2, space="PSUM"))
    psO = ctx.enter_context(tc.tile_pool(name="psO", bufs=1, space="PSUM"))

    # Weight DMAs on the scalar queue (hardware DGE), x-im2col on the sync
    # queue: the triggers issue in parallel at engine boot instead of
    # serializing behind one another. (gpsimd DMA = software DGE, slow.)
    w1_all = wpool.tile([128, 4 * COUT], BF16, tag="w1all", name="w1all")
    nc.scalar.dma_start(out=w1_all[:, :], in_=w1[:, :])
    w2_all = wpool.tile([CIN + 1, 4 * COUT], BF16, tag="w2all", name="w2all")
    nc.scalar.dma_start(out=w2_all[:, :], in_=w2[:, :])
    _ORD = {c: i for i, c in enumerate("qkvp")}
    w1_t = {c: w1_all[:, _ORD[c] * COUT : (_ORD[c] + 1) * COUT] for c in "qkvp"}
    w2_t = {c: w2_all[:, _ORD[c] * COUT : (_ORD[c] + 1) * COUT] for c in "qkvp"}

    def emit_xs(s, split=False):
        # im2col tiles.
        # xs1 rows 0-63  = x[cin, l-1]  (k=0), rows 64-127 = x[cin, l] (k=1)
        # xs2 rows 0-63  = x[cin, l+1]  (k=2), row 64 = ones (bias)
        # split=True: column-halved transfers, xs2 pieces on separate engine
        # queues so all triggers issue in parallel at boot.
        e2 = nc.scalar if split else nc.sync
        e3 = nc.sync
        xs1 = xpool.tile([128, L], BF16, tag="xs1", name="xs1")
        xs2 = xpool.tile([CIN + 1, L], BF16, tag="xs2", name="xs2")
        if split:
            nc.sync.dma_start(out=xs1[0:CIN, 1:HALF], in_=x[s, :, 0 : HALF - 1])
            nc.sync.dma_start(out=xs1[CIN:128, 0:HALF], in_=x[s, :, 0:HALF])
            nc.sync.dma_start(out=xs1[0:CIN, 0:1], in_=zc[:, :])
            e3.dma_start(out=xs1[0:CIN, HALF:L], in_=x[s, :, HALF - 1 : L - 1])
            e3.dma_start(out=xs1[CIN:128, HALF:L], in_=x[s, :, HALF:L])
            e2.dma_start(out=xs2[0:CIN, 0:HALF], in_=x[s, :, 1 : HALF + 1])
            e2.dma_start(out=xs2[CIN : CIN + 1, :], in_=onesrow[:, :])
            e2.dma_start(out=xs2[0:CIN, HALF : L - 1], in_=x[s, :, HALF + 1 : L])
            e2.dma_start(out=xs2[0:CIN, L - 1 : L], in_=zc[:, :])
        else:
            nc.sync.dma_start(out=xs1[0:CIN, 1:L], in_=x[s, :, 0 : L - 1])
            nc.sync.dma_start(out=xs1[0:CIN, 0:1], in_=zc[:, :])
            nc.sync.dma_start(out=xs1[CIN:128, 0:L], in_=x[s, :, :])
            nc.sync.dma_start(out=xs2[0:CIN, 0 : L - 1], in_=x[s, :, 1:L])
            nc.sync.dma_start(out=xs2[0:CIN, L - 1 : L], in_=zc[:, :])
            nc.sync.dma_start(out=xs2[CIN : CIN + 1, :], in_=onesrow[:, :])
        return xs1, xs2

    def lrelu_drain(dst_ap, ps_ap, mode):
        # lrelu(y) = max(y, slope*y)
        if mode == "scalar":
            # prelude only: ScalarE native Prelu, one op, DVE untouched.
            # (Lrelu's immediate alpha is ignored by the HW table; Prelu
            # reads the per-partition alpha AP correctly.)
            nc.scalar.activation(dst_ap, ps_ap, Prelu, alpha=alpha_t[:, :])
        else:
            # DVE 2-op: psum->sbuf bf16 copy (2x mode), then max(0.3*y, y).
            # (A single op reading PSUM twice is rejected: one PSUM read port.)
            w = ps_ap.free_size()
            yb = lpool.tile([128, w], BF16, tag="yb", name="yb")
            nc.vector.tensor_scalar_mul(yb[:, :], ps_ap, 1.0)
            nc.vector.scalar_tensor_tensor(
                dst_ap, yb[:, :], SLOPE, yb[:, :], op0=mult, op1=amax
            )

    def conv_q(xs1, xs2, p, dst, q, mode):
        # one [128,512] quarter of a [c, l]-layout conv (short PSUM-slot hold)
        cps = psA.tile([128, 512], F32, tag="ps", name="cps")
        c0 = q * 512
        nc.tensor.matmul(
            cps[:, :], w1_t[p], xs1[:, c0 : c0 + 512], start=True, stop=False
        )
        nc.tensor.matmul(
            cps[:, :], w2_t[p], xs2[:, c0 : c0 + 512], start=False, stop=True
        )
        lrelu_drain(dst[:, c0 : c0 + 512], cps[:, :], mode)

    def vt_qgroup(xs1, xs2, vt, gh, mode):
        # 4 l-blocks of V in transposed [l, c] layout -> one [128,512] tile
        vps = psA.tile([128, 512], F32, tag="ps", name="vps")
        for i in range(4):
            blk = gh * 4 + i
            lsl = slice(blk * 128, blk * 128 + 128)
            pc = slice(i * 128, i * 128 + 128)
            nc.tensor.matmul(
                vps[:, pc], xs1[:, lsl], w1_t["v"], start=True, stop=False
            )
            nc.tensor.matmul(
                vps[:, pc], xs2[:, lsl], w2_t["v"], start=False, stop=True
            )
        lrelu_drain(vt[:, gh * 512 : (gh + 1) * 512], vps[:, :], mode)

    def make_tiles():
        q_t = apool.tile([128, L], BF16, tag="actq", name="actq")
        k_t = apool.tile([128, L], BF16, tag="actk", name="actk")
        pe_t = apool.tile([128, L], BF16, tag="actp", name="actp")
        vt = apool.tile([128, L], BF16, tag="vt", name="vt")
        return q_t, k_t, pe_t, vt

    def attn_body(tiles, blk):
        """S matmuls + exp + normalization prep for one 128-row block.

        Z (softmax row sums) alternates per block between ScalarE's
        activation accumulator (costs 2 accumulator-read ops on the exp
        engine) and a fused DVE op over the bf16 P tile, spreading the Z
        cost across both engines."""
        q_t, k_t, pe_t, vt = tiles
        pblk = ppool.tile([128, L], BF16, tag="pblk", name="pblk")
        zz = zpool.tile([128, 2], F32, tag="zz", name="zz")
        for h in range(2):
            sps = psA.tile([128, HALF], F32, tag="ps", name="sps")
            for n in range(2):
                c0 = h * HALF + n * 512
                nc.tensor.matmul(
                    sps[:, n * 512 : n * 512 + 512],
                    q_t[:, blk * 128 : blk * 128 + 128],
                    k_t[:, c0 : c0 + 512],
                    start=True,
                    stop=True,
                )
            nc.scalar.activation(
                pblk[:, h * HALF : (h + 1) * HALF],
                sps[:, :],
                Exp,
                accum_out=zz[:, h : h + 1],
            )
        return pblk, zz

    def attn_norm(tiles, blk, zz):
        # z -> 1/z -> scaled v^T block; issued AFTER the iteration's filler
        # so filler drains (which free PSUM slots for the next S) come first
        # in the DVE queue. The 2-block out-matmul lag gives this chain slack.
        vt = tiles[3]
        z = zpool.tile([128, 1], F32, tag="z", name="z")
        nc.vector.tensor_tensor(z[:, :], zz[:, 0:1], zz[:, 1:2], mybir.AluOpType.add)
        r = zpool.tile([128, 1], F32, tag="r", name="r")
        nc.vector.reciprocal(r[:, :], z[:, :])
        vts = vpool.tile([128, 128], BF16, tag="vts", name="vts")
        nc.vector.tensor_scalar_mul(
            vts[:, :], vt[:, blk * 128 : blk * 128 + 128], r[:, :]
        )
        return vts

    def out_mms(out_ps, pblk, vts, blk):
        for n in range(4):
            nc.tensor.matmul(
                out_ps[:, n * 512 : n * 512 + 512],
                vts[:, :],
                pblk[:, n * 512 : n * 512 + 512],
                start=(blk == 0),
                stop=(blk == NB - 1),
            )

    def finish_sample(tiles, out_ps, s, last):
        # Interleaved drain: chunk n of the LAST block's out matmul is
        # followed immediately by its (psum + pe) -> bf16 add and DMA, so
        # the tail is one chunk deep instead of one sample deep.
        pe_t = tiles[2]
        pblk, vts, blk = last
        outs = opool.tile([128, L], BF16, tag="outs", name="outs")
        for n in range(4):
            cols = slice(n * 512, (n + 1) * 512)
            nc.tensor.matmul(
                out_ps[:, cols], vts[:, :], pblk[:, cols],
                start=(blk == 0), stop=True,
            )
            nc.vector.scalar_tensor_tensor(
                outs[:, cols], out_ps[:, cols], 1.0, pe_t[:, cols],
                op0=mult, op1=mybir.AluOpType.add,
            )
            nc.sync.dma_start(out=out[s, :, cols], in_=outs[:, cols])

    def attention_phase(tiles, out_ps, queue):
        """Software-pipelined over NB blocks. PE issue order per iteration:
        S(blk) matmuls first (feeding ScalarE), then out(blk-2) matmuls
        (the 2-block lag gives the z->recip->vts DVE chain slack), then
        deadline-due conv filler units. `queue` = [(deadline, thunk)]
        sorted by deadline; a unit with deadline d issues at the END of
        iteration d at the latest."""
        qi = 0
        pending = []
        for blk in range(NB):
            pblk, zz = attn_body(tiles, blk)
            while qi < len(queue) and queue[qi][0] <= blk:
                queue[qi][1]()
                qi += 1
            vts = attn_norm(tiles, blk, zz)
            pending.append((pblk, vts, blk))
            if len(pending) > 2:
                out_mms(out_ps, *pending.pop(0))
        while qi < len(queue):
            queue[qi][1]()
            qi += 1
        out_mms(out_ps, *pending.pop(0))
        return pending.pop(0)

    assert BP == 2
    # PE warm-up: dummy matmuls on a memset tile (no DMA dependency) keep
    # the PE busy from the start so the HAM clock-gate reaches 2.4 GHz
    # before the real work.
    wseed = wpool.tile([128, 128], BF16, tag="wseed", name="wseed")
    nc.gpsimd.memset(wseed[:, :], 0.001)
    alpha_t = wpool.tile([128, 1], F32, tag="alpha", name="alpha")
    nc.gpsimd.memset(alpha_t[:, :], SLOPE)
    wps = psA.tile([128, 128], F32, tag="ps", name="wps")
    for _ in range(NWARM):
        nc.tensor.matmul(
            wps[:, :], wseed[:, :], wseed[:, :], start=True, stop=True
        )
    # Prelude: only what attention block 0 needs -- all of K (moving side of
    # every S matmul) and Q quarter 0 (covers blocks 0-3). ScalarE-native
    # Lrelu drains (ScalarE is idle until the first exp).
    xs0 = emit_xs(0, split=True)
    tiles0 = make_tiles()
    q0, k0, pe0, vt0 = tiles0
    for q in range(4):
        conv_q(*xs0, "k", k0, q, "scalar")
    conv_q(*xs0, "q", q0, 0, "scalar")
    vt_qgroup(*xs0, vt0, 0, "scalar")
    # Phase B: sample-0 attention with remaining conv work deadline-dripped.
    # vt0 group g is read by out_mms(g*4) issued in iteration g*4+1, so its
    # deadline is g*4. q0 quarter j feeds S(4j) issued at iter 4j: deadline
    # 4j-1. Sample-1 q/k/vt group 0 are needed at phase-C start.
    xs1_ = emit_xs(1)
    tiles1 = make_tiles()
    q1, k1, pe1, vt1 = tiles1
    # Deadline rule (fillers issue at END of iteration d): a vt group g or
    # q quarter j consumed at iteration 4g must have deadline <= 4g-1, since
    # DVE/PE execute in issue order (a later-issued producer would deadlock).
    # At most ONE filler unit per iteration: double-filler iterations were
    # measured to open ~3us ScalarE bubbles at the group boundaries.
    queueB = [
        (0, lambda: conv_q(*xs0, "p", pe0, 0, "bf")),
        (1, lambda: conv_q(*xs0, "p", pe0, 1, "bf")),
        (2, lambda: vt_qgroup(*xs0, vt0, 1, "bf")),
        (3, lambda: conv_q(*xs0, "q", q0, 1, "bf")),
        (4, lambda: conv_q(*xs0, "p", pe0, 2, "bf")),
        (5, lambda: conv_q(*xs0, "p", pe0, 3, "bf")),
        (6, lambda: vt_qgroup(*xs0, vt0, 2, "bf")),
        (7, lambda: conv_q(*xs0, "q", q0, 2, "bf")),
        (8, lambda: conv_q(*xs1_, "k", k1, 0, "bf")),
        (9, lambda: conv_q(*xs1_, "k", k1, 1, "bf")),
        (10, lambda: vt_qgroup(*xs0, vt0, 3, "bf")),
        (11, lambda: conv_q(*xs0, "q", q0, 3, "bf")),
        (12, lambda: conv_q(*xs1_, "k", k1, 2, "bf")),
        (13, lambda: conv_q(*xs1_, "k", k1, 3, "bf")),
        (14, lambda: conv_q(*xs1_, "q", q1, 0, "bf")),
        (15, lambda: vt_qgroup(*xs1_, vt1, 0, "bf")),
    ]
    queueC = [
        (2, lambda: vt_qgroup(*xs1_, vt1, 1, "bf")),
        (3, lambda: conv_q(*xs1_, "q", q1, 1, "bf")),
        (5, lambda: conv_q(*xs1_, "p", pe1, 0, "bf")),
        (6, lambda: vt_qgroup(*xs1_, vt1, 2, "bf")),
        (7, lambda: conv_q(*xs1_, "q", q1, 2, "bf")),
        (9, lambda: conv_q(*xs1_, "p", pe1, 1, "bf")),
        (10, lambda: vt_qgroup(*xs1_, vt1, 3, "bf")),
        (11, lambda: conv_q(*xs1_, "q", q1, 3, "bf")),
        (13, lambda: conv_q(*xs1_, "p", pe1, 2, "bf")),
        (14, lambda: conv_q(*xs1_, "p", pe1, 3, "bf")),
    ]
    out_ps0 = psO.tile([128, L], F32, tag="ops", name="out_ps0")
    last0 = attention_phase(tiles0, out_ps0, queueB)
    finish_sample(tiles0, out_ps0, 0, last0)
    out_ps1 = psO.tile([128, L], F32, tag="ops", name="out_ps1")
    last1 = attention_phase(tiles1, out_ps1, queueC)
    finish_sample(tiles1, out_ps1, 1, last1)


def build():
    nc = bacc.Bacc("TRN2", target_bir_lowering=False, debug=False)
    x_d = nc.dram_tensor("x", [BP, CIN, L], BF16, kind="ExternalInput")
    w1_d = nc.dram_tensor("w1", [128, 4 * COUT], BF16, kind="ExternalInput")
    w2_d = nc.dram_tensor("w2", [CIN + 1, 4 * COUT], BF16, kind="ExternalInput")
    zc_d = nc.dram_tensor("zc", [CIN, 1], BF16, kind="ExternalInput")
    ones_d = nc.dram_tensor("onesrow", [1, L], BF16, kind="ExternalInput")
    out_d = nc.dram_tensor("out", [BP, COUT, L], BF16, kind="ExternalOutput")

    with tile.TileContext(nc) as tc, ExitStack() as ctx:
        _body(
            ctx,
            tc,
            x_d.ap(),
            w1_d.ap(),
            w2_d.ap(),
            zc_d.ap(),
            ones_d.ap(),
            out_d.ap(),
        )
    nc.compile()
    return nc


def _fold_weights(w, b, gamma, beta, mean, var):
    """Fold BN affine (fixed mean/var) into conv weights; return im2col chunks."""
    w = np.asarray(w, np.float64)
    scale = np.asarray(gamma, np.float64) / np.sqrt(np.asarray(var, np.float64) + EPS)
    shift = np.asarray(beta, np.float64) - np.asarray(mean, np.float64) * scale
    wf = w * scale[:, None, None]  # [COUT, CIN, K]
    bf = np.asarray(b, np.float64) * scale + shift
    w1 = np.empty((128, COUT), np.float32)
    w1[0:CIN] = wf[:, :, 0].T
    w1[CIN:128] = wf[:, :, 1].T
    w2 = np.empty((CIN + 1, COUT), np.float32)
    w2[0:CIN] = wf[:, :, 2].T
    w2[CIN] = bf
    return w1, w2


def _get_nc():
    if "nc" not in _CACHE:
        _CACHE["nc"] = build()
    return _CACHE["nc"]


def make_in_maps(inputs):
    bf = ml_dtypes.bfloat16
    x = np.ascontiguousarray(np.asarray(inputs["x"], np.float32).astype(bf))
    folded = {}
    for p in "qkvp":
        key = p if p != "p" else "pe"
        folded[p] = _fold_weights(
            inputs[f"{key}_w"],
            inputs[f"{key}_b"],
            inputs[f"{key}_gamma"],
            inputs[f"{key}_beta"],
            inputs[f"{key}_mean"],
            inputs[f"{key}_var"],
        )
    w1p = np.concatenate([folded[p][0] for p in "qkvp"], axis=1).astype(bf)
    w2p = np.concatenate([folded[p][1] for p in "qkvp"], axis=1).astype(bf)
    in_maps = []
    for i in range(NCORES):
        m = {"x": np.ascontiguousarray(x[i * BP : (i + 1) * BP])}
        m["w1"] = np.ascontiguousarray(w1p)
        m["w2"] = np.ascontiguousarray(w2p)
        m["zc"] = np.zeros((CIN, 1), bf)
        m["onesrow"] = np.ones((1, L), bf)
        in_maps.append(m)
    return in_maps


def kernel(**inputs):
    nc = _get_nc()
    in_maps = make_in_maps(inputs)
    res = run_bass_kernel_spmd(nc, in_maps, core_ids=list(range(NCORES)))
    out = np.concatenate([res.results[i]["out"] for i in range(NCORES)], axis=0)
    return out.astype(np.float32)


if __name__ == "__main__":
    rng = np.random.default_rng(0)
    ins = {"x": rng.standard_normal((B, CIN, L), dtype=np.float32)}
    for p in ("q", "k", "v", "pe"):
        ins[f"{p}_w"] = (rng.standard_normal((COUT, CIN, KW)) * 0.05).astype(np.float32)
        ins[f"{p}_b"] = (rng.standard_normal(COUT) * 0.05).astype(np.float32)
        ins[f"{p}_gamma"] = rng.uniform(0.5, 1.5, COUT).astype(np.float32)
        ins[f"{p}_beta"] = (rng.standard_normal(COUT) * 0.05).astype(np.float32)
        ins[f"{p}_mean"] = (rng.standard_normal(COUT) * 0.05).astype(np.float32)
        ins[f"{p}_var"] = rng.uniform(0.5, 1.5, COUT).astype(np.float32)
    got = kernel(**ins)
    print("kernel output:", got.shape, got.dtype, np.abs(got).mean())


# revision 24
# speedup vs baseline: 1.1000x; 1.0191x over previous
"""Trainium2 Bass kernel for nn_Conv1dAttention.

Math (per sample):
  q,k,v,pe = lrelu(bn(conv1d(x, W_p)))           # [C=128, L=2048], Cin=64, K=3
  S = q^T k                                      # [L, L]
  P = softmax_rows(S)                            # softmax over last axis
  out = v @ P + pe                               # [C, L]

Sharding: data-parallel over batch B=16 across 8 NeuronCores (2 samples/core).
Same NEFF on all cores, per-core input shards, no collectives.

Design notes:
  - BN (uses given mean/var, not data stats) is folded into conv weights and
    bias on the host. Bias is injected via an appended ones-row in the im2col
    tile, so conv+bias is pure matmul.
  - im2col: contraction 192 = two chunks: chunk1 = 128 rows (k=0 shifted,
    k=1 center), chunk2 = 65 rows (k=2 shifted + ones row for bias).
  - Q, K, PE computed in [c, l] layout; V directly transposed [l, c].
  - bf16 matmul operands everywhere; PSUM accumulation fp32.
  - ScalarE (exp) is the roofline engine (~73us of EXP per core); the
    schedule keeps it saturated: prelude computes only what the first S
    block needs (k all, q quarter 0), everything else is deadline-paced
    PE filler inside the two attention phases.
  - Z (softmax row sums) via a DVE tensor_scalar copy-with-accumulator over
    the bf16 P tile (4x mode), keeping Scalar at its pure-exp floor.
  - LReLU drains: one DVE scalar_tensor_tensor op reading PSUM twice
    (max(0.3*y, y)); prelude drains use ScalarE's native Lrelu (idle then).
  - Output stored bf16 (halves the out DMA), cast to fp32 on host.
  - PSUM: 4 banks out accumulator; 4 banks rotating S/conv tiles.
"""

import sys

if "/opt/trn_rl_repo" not in sys.path:
    sys.path.insert(0, "/opt/trn_rl_repo")

from contextlib import ExitStack

import ml_dtypes
import numpy as np

import concourse.bass as bass
import concourse.tile as tile
from concourse import bacc, mybir
from concourse.bass_utils import run_bass_kernel_spmd

B, CIN, COUT, KW, L = 16, 64, 128, 3, 2048
NCORES = 8
BP = B // NCORES  # samples per core
EPS = 1e-5
SLOPE = 0.3
F32 = mybir.dt.float32
BF16 = mybir.dt.bfloat16
NB = L // 128  # 16 a-blocks
HALF = 1024
NWARM = 26

_CACHE = {}


def _body(ctx, tc, x, w1, w2, zc, onesrow, out):
    nc = tc.nc
    amax = mybir.AluOpType.max
    mult = mybir.AluOpType.mult
    Exp = mybir.ActivationFunctionType.Exp
    Prelu = mybir.ActivationFunctionType.Prelu

    wpool = ctx.enter_context(tc.tile_pool(name="wpool", bufs=1))
    xpool = ctx.enter_context(tc.tile_pool(name="xpool", bufs=2))
    apool = ctx.enter_context(tc.tile_pool(name="apool", bufs=2))
    ppool = ctx.enter_context(tc.tile_pool(name="ppool", bufs=4))
    opool = ctx.enter_context(tc.tile_pool(name="opool", bufs=2))
    vpool = ctx.enter_context(tc.tile_pool(name="vpool", bufs=4))
    zpool = ctx.enter_context(tc.tile_pool(name="zpool", bufs=4))
    lpool = ctx.enter_context(tc.tile_pool(name="lpool", bufs=2))
    psA = ctx.enter_context(tc.tile_pool(name="psA", bufs=2, space="PSUM"))
    psO = ctx.enter_context(tc.tile_pool(name="psO", bufs=1, space="PSUM"))

    # Weight DMAs on the scalar queue (hardware DGE), x-im2col on the sync
    # queue: the triggers issue in parallel at engine boot instead of
    # serializing behind one another. (gpsimd DMA = software DGE, slow.)
    w1_all = wpool.tile([128, 4 * COUT], BF16, tag="w1all", name="w1all")
    nc.scalar.dma_start(out=w1_all[:, :], in_=w1[:, :])
    w2_all = wpool.tile([CIN + 1, 4 * COUT], BF16, tag="w2all", name="w2all")
    nc.scalar.dma_start(out=w2_all[:, :], in_=w2[:, :])
    _ORD = {c: i for i, c in enumerate("qkvp")}
    w1_t = {c: w1_all[:, _ORD[c] * COUT : (_ORD[c] + 1) * COUT] for c in "qkvp"}
    w2_t = {c: w2_all[:, _ORD[c] * COUT : (_ORD[c] + 1) * COUT] for c in "qkvp"}

    def emit_xs(s, split=False):
        # im2col tiles.
        # xs1 rows 0-63  = x[cin, l-1]  (k=0), rows 64-127 = x[cin, l] (k=1)
        # xs2 rows 0-63  = x[cin, l+1]  (k=2), row 64 = ones (bias)
        # split=True: column-halved transfers, xs2 pieces on separate engine
        # queues so all triggers issue in parallel at boot.
        e2 = nc.scalar if split else nc.sync
        e3 = nc.sync
        xs1 = xpool.tile([128, L], BF16, tag="xs1", name="xs1")
        xs2 = xpool.tile([CIN + 1, L], BF16, tag="xs2", name="xs2")
        if split:
            nc.sync.dma_start(out=xs1[0:CIN, 1:HALF], in_=x[s, :, 0 : HALF - 1])
            nc.sync.dma_start(out=xs1[CIN:128, 0:HALF], in_=x[s, :, 0:HALF])
            nc.sync.dma_start(out=xs1[0:CIN, 0:1], in_=zc[:, :])
            e3.dma_start(out=xs1[0:CIN, HALF:L], in_=x[s, :, HALF - 1 : L - 1])
            e3.dma_start(out=xs1[CIN:128, HALF:L], in_=x[s, :, HALF:L])
            e2.dma_start(out=xs2[0:CIN, 0:HALF], in_=x[s, :, 1 : HALF + 1])
            e2.dma_start(out=xs2[CIN : CIN + 1, :], in_=onesrow[:, :])
            e2.dma_start(out=xs2[0:CIN, HALF : L - 1], in_=x[s, :, HALF + 1 : L])
            e2.dma_start(out=xs2[0:CIN, L - 1 : L], in_=zc[:, :])
        else:
            nc.sync.dma_start(out=xs1[0:CIN, 1:L], in_=x[s, :, 0 : L - 1])
            nc.sync.dma_start(out=xs1[0:CIN, 0:1], in_=zc[:, :])
            nc.sync.dma_start(out=xs1[CIN:128, 0:L], in_=x[s, :, :])
            nc.sync.dma_start(out=xs2[0:CIN, 0 : L - 1], in_=x[s, :, 1:L])
            nc.sync.dma_start(out=xs2[0:CIN, L - 1 : L], in_=zc[:, :])
            nc.sync.dma_start(out=xs2[CIN : CIN + 1, :], in_=onesrow[:, :])
        return xs1, xs2

    def lrelu_drain(dst_ap, ps_ap, mode):
        # lrelu(y) = max(y, slope*y)
        if mode == "scalar":
            # prelude only: ScalarE native Prelu, one op, DVE untouched.
            # (Lrelu's immediate alpha is ignored by the HW table; Prelu
            # reads the per-partition alpha AP correctly.)
            nc.scalar.activation(dst_ap, ps_ap, Prelu, alpha=alpha_t[:, :])
        else:
            # DVE 2-op: psum->sbuf bf16 copy (2x mode), then max(0.3*y, y).
            # (A single op reading PSUM twice is rejected: one PSUM read port.)
            w = ps_ap.free_size()
            yb = lpool.tile([128, w], BF16, tag="yb", name="yb")
            nc.vector.tensor_scalar_mul(yb[:, :], ps_ap, 1.0)
            nc.vector.scalar_tensor_tensor(
                dst_ap, yb[:, :], SLOPE, yb[:, :], op0=mult, op1=amax
            )

    def conv_q(xs1, xs2, p, dst, q, mode):
        # one [128,512] quarter of a [c, l]-layout conv (short PSUM-slot hold)
        cps = psA.tile([128, 512], F32, tag="ps", name="cps")
        c0 = q * 512
        nc.tensor.matmul(
            cps[:, :], w1_t[p], xs1[:, c0 : c0 + 512], start=True, stop=False
        )
        nc.tensor.matmul(
            cps[:, :], w2_t[p], xs2[:, c0 : c0 + 512], start=False, stop=True
        )
        lrelu_drain(dst[:, c0 : c0 + 512], cps[:, :], mode)

    def vt_qgroup(xs1, xs2, vt, gh, mode, nb=4):
        # nb l-blocks of V in transposed [l, c] layout -> one [128,nb*128]
        # tile. nb=2 halves the filler burst per attention iteration.
        vps = psA.tile([128, nb * 128], F32, tag="ps", name="vps")
        for i in range(nb):
            blk = gh * nb + i
            lsl = slice(blk * 128, blk * 128 + 128)
            pc = slice(i * 128, i * 128 + 128)
            nc.tensor.matmul(
                vps[:, pc], xs1[:, lsl], w1_t["v"], start=True, stop=False
            )
            nc.tensor.matmul(
                vps[:, pc], xs2[:, lsl], w2_t["v"], start=False, stop=True
            )
        lrelu_drain(
            vt[:, gh * nb * 128 : (gh + 1) * nb * 128], vps[:, :], mode
        )

    def make_tiles():
        q_t = apool.tile([128, L], BF16, tag="actq", name="actq")
        k_t = apool.tile([128, L], BF16, tag="actk", name="actk")
        pe_t = apool.tile([128, L], BF16, tag="actp", name="actp")
        vt = apool.tile([128, L], BF16, tag="vt", name="vt")
        return q_t, k_t, pe_t, vt

    def attn_body(tiles, blk):
        """S matmuls + exp + normalization prep for one 128-row block.

        Z (softmax row sums) alternates per block between ScalarE's
        activation accumulator (costs 2 accumulator-read ops on the exp
        engine) and a fused DVE op over the bf16 P tile, spreading the Z
        cost across both engines."""
        q_t, k_t, pe_t, vt = tiles
        z_on_dve = blk % 2 == 1
        pblk = ppool.tile([128, L], BF16, tag="pblk", name="pblk")
        zz = zpool.tile([128, 2], F32, tag="zz", name="zz")
        for h in range(2):
            sps = psA.tile([128, HALF], F32, tag="ps", name="sps")
            for n in range(2):
                c0 = h * HALF + n * 512
                nc.tensor.matmul(
                    sps[:, n * 512 : n * 512 + 512],
                    q_t[:, blk * 128 : blk * 128 + 128],
                    k_t[:, c0 : c0 + 512],
                    start=True,
                    stop=True,
                )
            nc.scalar.activation(
                pblk[:, h * HALF : (h + 1) * HALF],
                sps[:, :],
                Exp,
                accum_out=None if z_on_dve else zz[:, h : h + 1],
            )
        return pblk, zz

    def attn_norm(tiles, blk, zz, pblk):
        # z -> 1/z -> scaled v^T block; issued AFTER the iteration's filler
        # so filler drains (which free PSUM slots for the next S) come first
        # in the DVE queue. The 2-block out-matmul lag gives this chain slack.
        vt = tiles[3]
        z = zpool.tile([128, 1], F32, tag="z", name="z")
        if blk % 2 == 1:
            zscr = lpool.tile([128, HALF], BF16, tag="zscr", name="zscr")
            nc.vector.scalar_tensor_tensor(
                zscr[:, :], pblk[:, 0:HALF], 1.0, pblk[:, HALF:L],
                op0=mult, op1=mybir.AluOpType.add, accum_out=z[:, :],
            )
        else:
            nc.vector.tensor_tensor(
                z[:, :], zz[:, 0:1], zz[:, 1:2], mybir.AluOpType.add
            )
        r = zpool.tile([128, 1], F32, tag="r", name="r")
        nc.vector.reciprocal(r[:, :], z[:, :])
        vts = vpool.tile([128, 128], BF16, tag="vts", name="vts")
        nc.vector.tensor_scalar_mul(
            vts[:, :], vt[:, blk * 128 : blk * 128 + 128], r[:, :]
        )
        return vts

    def out_mms(out_ps, pblk, vts, blk):
        for n in range(4):
            nc.tensor.matmul(
                out_ps[:, n * 512 : n * 512 + 512],
                vts[:, :],
                pblk[:, n * 512 : n * 512 + 512],
                start=(blk == 0),
                stop=(blk == NB - 1),
            )

    def finish_sample(tiles, out_ps, s, last):
        # Interleaved drain: chunk n of the LAST block's out matmul is
        # followed immediately by its (psum + pe) -> bf16 add and DMA, so
        # the tail is one chunk deep instead of one sample deep.
        pe_t = tiles[2]
        pblk, vts, blk = last
        outs = opool.tile([128, L], BF16, tag="outs", name="outs")
        for n in range(4):
            cols = slice(n * 512, (n + 1) * 512)
            nc.tensor.matmul(
                out_ps[:, cols], vts[:, :], pblk[:, cols],
                start=(blk == 0), stop=True,
            )
            nc.vector.scalar_tensor_tensor(
                outs[:, cols], out_ps[:, cols], 1.0, pe_t[:, cols],
                op0=mult, op1=mybir.AluOpType.add,
            )
            nc.sync.dma_start(out=out[s, :, cols], in_=outs[:, cols])

    def attention_phase(tiles, out_ps, queue):
        """Software-pipelined over NB blocks. PE issue order per iteration:
        S(blk) matmuls first (feeding ScalarE), then out(blk-2) matmuls
        (the 2-block lag gives the z->recip->vts DVE chain slack), then
        deadline-due conv filler units. `queue` = [(deadline, thunk)]
        sorted by deadline; a unit with deadline d issues at the END of
        iteration d at the latest."""
        qi = 0
        pending = []
        for blk in range(NB):
            pblk, zz = attn_body(tiles, blk)
            while qi < len(queue) and queue[qi][0] <= blk:
                queue[qi][1]()
                qi += 1
            vts = attn_norm(tiles, blk, zz, pblk)
            pending.append((pblk, vts, blk))
            if len(pending) > 2:
                out_mms(out_ps, *pending.pop(0))
        while qi < len(queue):
            queue[qi][1]()
            qi += 1
        out_mms(out_ps, *pending.pop(0))
        return pending.pop(0)

    assert BP == 2
    # PE warm-up: dummy matmuls on a memset tile (no DMA dependency) keep
    # the PE busy from the start so the HAM clock-gate reaches 2.4 GHz
    # before the real work.
    wseed = wpool.tile([128, 128], BF16, tag="wseed", name="wseed")
    nc.gpsimd.memset(wseed[:, :], 0.001)
    alpha_t = wpool.tile([128, 1], F32, tag="alpha", name="alpha")
    nc.gpsimd.memset(alpha_t[:, :], SLOPE)
    wps = psA.tile([128, 128], F32, tag="ps", name="wps")
    for _ in range(NWARM):
        nc.tensor.matmul(
            wps[:, :], wseed[:, :], wseed[:, :], start=True, stop=True
        )
    # Prelude: only what attention block 0 needs -- all of K (moving side of
    # every S matmul) and Q quarter 0 (covers blocks 0-3). ScalarE-native
    # Lrelu drains (ScalarE is idle until the first exp).
    xs0 = emit_xs(0, split=True)
    tiles0 = make_tiles()
    q0, k0, pe0, vt0 = tiles0
    for q in range(4):
        conv_q(*xs0, "k", k0, q, "scalar")
    conv_q(*xs0, "q", q0, 0, "scalar")
    vt_qgroup(*xs0, vt0, 0, "scalar", nb=2)
    # Phase B: sample-0 attention with remaining conv work deadline-dripped.
    # vt0 group g is read by out_mms(g*4) issued in iteration g*4+1, so its
    # deadline is g*4. q0 quarter j feeds S(4j) issued at iter 4j: deadline
    # 4j-1. Sample-1 q/k/vt group 0 are needed at phase-C start.
    xs1_ = emit_xs(1)
    tiles1 = make_tiles()
    q1, k1, pe1, vt1 = tiles1
    # Deadline rule (fillers issue at END of iteration d): a vt group g or
    # q quarter j consumed at iteration 4g must have deadline <= 4g-1, since
    # DVE/PE execute in issue order (a later-issued producer would deadlock).
    # At most ONE filler unit per iteration: double-filler iterations were
    # measured to open ~3us ScalarE bubbles at the group boundaries.
    def vth(xs, vt, hg):
        return lambda: vt_qgroup(*xs, vt, hg, "bf", nb=2)

    queueB = [
        (0, lambda: conv_q(*xs0, "p", pe0, 0, "bf")),
        (0, vth(xs0, vt0, 1)),
        (1, lambda: conv_q(*xs0, "p", pe0, 1, "bf")),
        (1, vth(xs0, vt0, 2)),
        (2, lambda: conv_q(*xs0, "q", q0, 1, "bf")),
        (3, vth(xs0, vt0, 3)),
        (4, lambda: conv_q(*xs0, "p", pe0, 2, "bf")),
        (5, vth(xs0, vt0, 4)),
        (6, lambda: conv_q(*xs0, "q", q0, 2, "bf")),
        (7, vth(xs0, vt0, 5)),
        (8, lambda: conv_q(*xs1_, "k", k1, 0, "bf")),
        (9, vth(xs0, vt0, 6)),
        (10, lambda: conv_q(*xs0, "q", q0, 3, "bf")),
        (11, vth(xs0, vt0, 7)),
        (12, lambda: conv_q(*xs1_, "k", k1, 1, "bf")),
        (13, lambda: conv_q(*xs1_, "k", k1, 2, "bf")),
        (14, lambda: conv_q(*xs1_, "k", k1, 3, "bf")),
        (14, lambda: conv_q(*xs0, "p", pe0, 3, "bf")),
        (15, lambda: conv_q(*xs1_, "q", q1, 0, "bf")),
        (15, vth(xs1_, vt1, 0)),
    ]
    queueC = [
        (0, vth(xs1_, vt1, 1)),
        (1, vth(xs1_, vt1, 2)),
        (2, lambda: conv_q(*xs1_, "q", q1, 1, "bf")),
        (3, vth(xs1_, vt1, 3)),
        (4, lambda: conv_q(*xs1_, "p", pe1, 0, "bf")),
        (5, vth(xs1_, vt1, 4)),
        (6, lambda: conv_q(*xs1_, "q", q1, 2, "bf")),
        (7, vth(xs1_, vt1, 5)),
        (8, lambda: conv_q(*xs1_, "p", pe1, 1, "bf")),
        (9, vth(xs1_, vt1, 6)),
        (10, lambda: conv_q(*xs1_, "q", q1, 3, "bf")),
        (11, vth(xs1_, vt1, 7)),
        (12, lambda: conv_q(*xs1_, "p", pe1, 2, "bf")),
        (13, lambda: conv_q(*xs1_, "p", pe1, 3, "bf")),
    ]
    out_ps0 = psO.tile([128, L], F32, tag="ops", name="out_ps0")
    last0 = attention_phase(tiles0, out_ps0, queueB)
    finish_sample(tiles0, out_ps0, 0, last0)
    out_ps1 = psO.tile([128, L], F32, tag="ops", name="out_ps1")
    last1 = attention_phase(tiles1, out_ps1, queueC)
    finish_sample(tiles1, out_ps1, 1, last1)


def build():
    nc = bacc.Bacc("TRN2", target_bir_lowering=False, debug=False)
    x_d = nc.dram_tensor("x", [BP, CIN, L], BF16, kind="ExternalInput")
    w1_d = nc.dram_tensor("w1", [128, 4 * COUT], BF16, kind="ExternalInput")
    w2_d = nc.dram_tensor("w2", [CIN + 1, 4 * COUT], BF16, kind="ExternalInput")
    zc_d = nc.dram_tensor("zc", [CIN, 1], BF16, kind="ExternalInput")
    ones_d = nc.dram_tensor("onesrow", [1, L], BF16, kind="ExternalInput")
    out_d = nc.dram_tensor("out", [BP, COUT, L], BF16, kind="ExternalOutput")

    with tile.TileContext(nc) as tc, ExitStack() as ctx:
        _body(
            ctx,
            tc,
            x_d.ap(),
            w1_d.ap(),
            w2_d.ap(),
            zc_d.ap(),
            ones_d.ap(),
            out_d.ap(),
        )
    nc.compile()
    return nc


def _fold_weights(w, b, gamma, beta, mean, var):
    """Fold BN affine (fixed mean/var) into conv weights; return im2col chunks."""
    w = np.asarray(w, np.float64)
    scale = np.asarray(gamma, np.float64) / np.sqrt(np.asarray(var, np.float64) + EPS)
    shift = np.asarray(beta, np.float64) - np.asarray(mean, np.float64) * scale
    wf = w * scale[:, None, None]  # [COUT, CIN, K]
    bf = np.asarray(b, np.float64) * scale + shift
    w1 = np.empty((128, COUT), np.float32)
    w1[0:CIN] = wf[:, :, 0].T
    w1[CIN:128] = wf[:, :, 1].T
    w2 = np.empty((CIN + 1, COUT), np.float32)
    w2[0:CIN] = wf[:, :, 2].T
    w2[CIN] = bf
    return w1, w2


def _get_nc():
    if "nc" not in _CACHE:
        _CACHE["nc"] = build()
    return _CACHE["nc"]


def make_in_maps(inputs):
    bf = ml_dtypes.bfloat16
    x = np.ascontiguousarray(np.asarray(inputs["x"], np.float32).astype(bf))
    folded = {}
    for p in "qkvp":
        key = p if p != "p" else "pe"
        folded[p] = _fold_weights(
            inputs[f"{key}_w"],
            inputs[f"{key}_b"],
            inputs[f"{key}_gamma"],
            inputs[f"{key}_beta"],
            inputs[f"{key}_mean"],
            inputs[f"{key}_var"],
        )
    w1p = np.concatenate([folded[p][0] for p in "qkvp"], axis=1).astype(bf)
    w2p = np.concatenate([folded[p][1] for p in "qkvp"], axis=1).astype(bf)
    in_maps = []
    for i in range(NCORES):
        m = {"x": np.ascontiguousarray(x[i * BP : (i + 1) * BP])}
        m["w1"] = np.ascontiguousarray(w1p)
        m["w2"] = np.ascontiguousarray(w2p)
        m["zc"] = np.zeros((CIN, 1), bf)
        m["onesrow"] = np.ones((1, L), bf)
        in_maps.append(m)
    return in_maps


def kernel(**inputs):
    nc = _get_nc()
    in_maps = make_in_maps(inputs)
    res = run_bass_kernel_spmd(nc, in_maps, core_ids=list(range(NCORES)))
    out = np.concatenate([res.results[i]["out"] for i in range(NCORES)], axis=0)
    return out.astype(np.float32)


if __name__ == "__main__":
    rng = np.random.default_rng(0)
    ins = {"x": rng.standard_normal((B, CIN, L), dtype=np.float32)}
    for p in ("q", "k", "v", "pe"):
        ins[f"{p}_w"] = (rng.standard_normal((COUT, CIN, KW)) * 0.05).astype(np.float32)
        ins[f"{p}_b"] = (rng.standard_normal(COUT) * 0.05).astype(np.float32)
        ins[f"{p}_gamma"] = rng.uniform(0.5, 1.5, COUT).astype(np.float32)
        ins[f"{p}_beta"] = (rng.standard_normal(COUT) * 0.05).astype(np.float32)
        ins[f"{p}_mean"] = (rng.standard_normal(COUT) * 0.05).astype(np.float32)
        ins[f"{p}_var"] = rng.uniform(0.5, 1.5, COUT).astype(np.float32)
    got = kernel(**ins)
    print("kernel output:", got.shape, got.dtype, np.abs(got).mean())
